# revision 33
# baseline (speedup 1.0000x reference)
"""Self-contained TRN2 Bass kernel for nn_MultiHeadAttention_77833397338481.

kernel(**inputs) takes the FULL unsharded inputs (Q, K, V [2,1024,1024],
Wq/Wk/Wv/Wo [1024,1024], biases [1024]) and returns the FULL output
[2, 1024, 1024]. 8 NeuronCores = batch(2) x head-group(4).

Per-core pipeline (balanced so the PE stays dense/warm while the scalar
engine's exp stream is the critical path):
  - q/k projections via fp8e4 DoubleRow matmuls (256-deep contraction per
    instruction), evacuated to packed f32r qt/kt (16 dims per 32-row band)
  - scores = k^T q per head as f32r matmuls with PE tile_position packing
  - exp on the scalar engine (PSUM f32 -> SBUF bf16), ones-column trick for
    softmax denominators, denominator broadcast via a select matmul
  - ctx and output projection in bf16; per-core partials summed on host
"""

import numpy as np
import ml_dtypes

import concourse.bacc as bacc
import concourse.mybir as mybir
import concourse.tile as tile

F32 = mybir.dt.float32
F32R = mybir.dt.float32r
BF16 = mybir.dt.bfloat16
FP8 = mybir.dt.float8e4
AF = mybir.ActivationFunctionType
DR = mybir.MatmulPerfMode.DoubleRow
ADD = mybir.AluOpType.add

NP_FP8 = ml_dtypes.float8_e4m3fn
NP_BF16 = ml_dtypes.bfloat16

D = 1024
S = 1024
B = 2
E = 16      # per-head dim
NHQ = 4     # heads per quad
NQUAD = 4   # quads per core
VW = 16 * 17  # 272: 16 heads x (16 v-dims + ones col)
SCALE = 1.0 / 32.0


def round_fp32r(x):
    u = np.ascontiguousarray(x, np.float32).view(np.uint32)
    r = ((u.astype(np.uint64) + 0x800) & 0xFFFFF000).astype(np.uint32)
    return r.view(np.float32)


def build_nc():
    nc = bacc.Bacc("TRN2", target_bir_lowering=False, debug=False, num_devices=8)

    xq_d = nc.dram_tensor("xq", [512, 2, 1024], FP8, kind="ExternalInput")
    xk_d = nc.dram_tensor("xk", [512, 2, 1024], FP8, kind="ExternalInput")
    xv_d = nc.dram_tensor("xv", [128, 8, 1024], BF16, kind="ExternalInput")
    wq_d = nc.dram_tensor("wq", [128, 16, 256], FP8, kind="ExternalInput")
    wk_d = nc.dram_tensor("wk", [128, 16, 256], FP8, kind="ExternalInput")
    wv_d = nc.dram_tensor("wv", [128, 8, VW], BF16, kind="ExternalInput")
    wot_d = nc.dram_tensor("wot", [128, 4, 1024], BF16, kind="ExternalInput")
    bq_d = nc.dram_tensor("bqp", [128, 4], F32, kind="ExternalInput")
    bk_d = nc.dram_tensor("bkp", [128, 4], F32, kind="ExternalInput")
    bvrow_d = nc.dram_tensor("bvrow", [1, VW], F32, kind="ExternalInput")
    sel_d = nc.dram_tensor("sel", [128, 128], F32R, kind="ExternalInput")
    out_d = nc.dram_tensor("out_part", [S, D], F32, kind="ExternalOutput")

    with tile.TileContext(nc) as tc:
        with (
            tc.tile_pool(name="persist", bufs=1) as pp,
            tc.tile_pool(name="attn", bufs=1) as ap_,
            tc.tile_pool(name="psum", space="PSUM", bufs=1) as ps,
        ):
            # --- warm the exp table ASAP ---
            dummy = pp.tile([1, 8], F32, name="dummy")
            nc.vector.memset(dummy, 0.0)
            dummy2 = pp.tile([1, 8], F32, name="dummy2")
            nc.scalar.activation(dummy2, dummy, AF.Exp)

            # --- constants ---
            sel_sb = pp.tile([128, 128], F32R, name="sel_sb")
            nc.sync.dma_start(out=sel_sb, in_=sel_d[:, :])
            bvrow_sb = pp.tile([1, VW], F32, name="bvrow_sb")
            nc.sync.dma_start(out=bvrow_sb, in_=bvrow_d[:, :])
            bq_sb = pp.tile([128, 4], F32, name="bq_sb")
            nc.sync.dma_start(out=bq_sb, in_=bq_d[:, :])
            bk_sb = pp.tile([128, 4], F32, name="bk_sb")
            nc.sync.dma_start(out=bk_sb, in_=bk_d[:, :])
            ones1 = pp.tile([1, 128], F32, name="ones1")
            nc.vector.memset(ones1, 1.0)

            # --- persistent tiles ---
            wq_all = pp.tile([128, 16, 256], FP8, name="wq_all")
            wk_all = pp.tile([128, 16, 256], FP8, name="wk_all")
            xq_sb = [pp.tile([128, 2, 1024], FP8, name=f"xq{b}") for b in range(4)]
            xk_sb = [pp.tile([128, 2, 1024], FP8, name=f"xk{b}") for b in range(4)]
            xv_all = pp.tile([128, 8, 1024], BF16, name="xv_all")
            wv_all = pp.tile([128, 8, VW], BF16, name="wv_all")
            wot_all = pp.tile([128, 4, 1024], BF16, name="wot_all")
            qt = [pp.tile([128, 1024], F32R, name=f"qt{t}") for t in range(NQUAD)]
            kt = [pp.tile([128, 1024], F32R, name=f"kt{t}") for t in range(NQUAD)]
            va = [pp.tile([128, VW], BF16, name=f"va{c}") for c in range(8)]
            # ctxp[t] rows 32j+e (e<16) hold head (t,j)'s normalized ctx; pad
            # rows must be zero (outproj contracts all 128 partitions)
            ctxp = [pp.tile([128, 1024], BF16, name=f"ctxp{t}") for t in range(NQUAD)]
            for t in range(NQUAD):
                nc.vector.memset(ctxp[t].bitcast(mybir.dt.uint16), 0)
            # persistent stage buffers, pre-zeroed (pad rows stay zero; the
            # select matmul contracts over them)
            st_bufs = [pp.tile([128, 512], F32R, name=f"stb{z}") for z in range(2)]
            for z in range(2):
                nc.vector.memset(st_bufs[z].bitcast(mybir.dt.uint32), 0)

            # --- input DMAs, in consumption order ---
            nc.sync.dma_start(out=wq_all, in_=wq_d[:, :, :])
            for b in range(4):
                nc.sync.dma_start(out=xq_sb[b], in_=xq_d[128 * b : 128 * (b + 1), :, :])
            nc.sync.dma_start(out=wk_all, in_=wk_d[:, :, :])
            for b in range(4):
                nc.sync.dma_start(out=xk_sb[b], in_=xk_d[128 * b : 128 * (b + 1), :, :])
            nc.sync.dma_start(out=wv_all, in_=wv_d[:, :, :])
            for c in range(8):
                nc.sync.dma_start(
                    out=xv_all[:, :, 128 * c : 128 * (c + 1)],
                    in_=xv_d[:, :, 128 * c : 128 * (c + 1)],
                )
            nc.sync.dma_start(out=wot_all, in_=wot_d[:, :, :])

            # --- q/k projection half: fp8 DoubleRow over 4 superblocks into
            # one aux bank, evacuated to packed f32r qt/kt ---
            def qk_proj_half(t, h, w_all, xs, dst, bias_sb, nm, tag="aux"):
                sc = ps.tile([128, 512], F32, name=f"p{nm}{t}{h}", tag=tag, bufs=1)
                for b in range(4):
                    wt = w_all[:, 4 * t + b : 4 * t + b + 1, :].rearrange(
                        "p o (pair m) -> p (o pair) m", pair=2
                    )
                    nc.tensor.matmul(
                        sc,
                        wt,
                        xs[b][:, :, 512 * h : 512 * (h + 1)],
                        start=(b == 0),
                        stop=(b == 3),
                        perf_mode=DR,
                    )
                nc.vector.tensor_scalar(
                    dst[t][:, 512 * h : 512 * (h + 1)],
                    sc,
                    bias_sb[:, t : t + 1],
                    None,
                    ADD,
                )

            # --- biasB = broadcast of bvrow over 128 partitions ---
            biasB_ps = ps.tile([128, VW], F32, name="biasB_ps", tag="sc", bufs=3)
            nc.tensor.matmul(biasB_ps, ones1, bvrow_sb, start=True, stop=True)
            biasB = pp.tile([128, VW], F32, name="biasB")
            nc.vector.tensor_copy(biasB, biasB_ps)

            # quad 0 h0 projections first so attention starts ASAP (h1
            # halves are spread into the first window's blocks)
            qk_proj_half(0, 0, wq_all, xq_sb, qt, bq_sb, "q")
            qk_proj_half(0, 0, wk_all, xk_sb, kt, bk_sb, "k")

            # --- v projection for one sk-block (spread into attention) ---
            def v_proj(c, tag="aux"):
                pv = ps.tile([128, VW], F32, name=f"pv{c}", tag=tag, bufs=1)
                for d in range(8):
                    nc.tensor.matmul(
                        pv,
                        xv_all[:, d : d + 1, 128 * c : 128 * (c + 1)],
                        wv_all[:, d : d + 1, :],
                        start=(d == 0),
                        stop=(d == 7),
                    )
                nc.vector.tensor_add(va[c], pv, biasB)


            # ================= attention =================
            def outproj_group(m, dc, tag="aux", evac="vector"):
                po = ps.tile(
                    [128, 512], F32, name=f"po{m}{dc}", tag=tag,
                    bufs=3 if tag == "sc" else 1,
                )
                for t_ in range(NQUAD):
                    nc.tensor.matmul(
                        po,
                        ctxp[t_][:, 128 * m : 128 * (m + 1)],
                        wot_all[:, t_ : t_ + 1, 512 * dc : 512 * (dc + 1)],
                        start=(t_ == 0),
                        stop=(t_ == NQUAD - 1),
                    )
                og = ap_.tile([128, 512], F32, name=f"og{m}{dc}", tag="og", bufs=4)
                if evac == "scalar":
                    nc.scalar.copy(og, po)
                else:
                    nc.vector.tensor_copy(og, po)
                nc.sync.dma_start(
                    out=out_d[128 * m : 128 * (m + 1), 512 * dc : 512 * (dc + 1)],
                    in_=og,
                )

            # projection-piece placement: (window t, n, block c) -> (quad, half, q/k)
            PROJ_FILLS = {
                (0, 0, 1): (0, 1, "k"),
                (0, 0, 2): (0, 1, "q"),
                (0, 1, 1): (1, 0, "q"),
                (0, 1, 3): (1, 0, "k"),
                (0, 1, 5): (1, 1, "k"),
                (1, 0, 1): (1, 1, "q"),
                (1, 0, 3): (2, 0, "q"),
                (1, 1, 1): (2, 0, "k"),
                (1, 1, 5): (2, 1, "k"),
                (2, 0, 1): (2, 1, "q"),
                (2, 0, 3): (3, 0, "q"),
                (2, 1, 1): (3, 0, "k"),
                (2, 1, 5): (3, 1, "k"),
                (3, 0, 1): (3, 1, "q"),
            }

            stv = 0
            for t in range(NQUAD):
                for n in range(2):
                    ctx_t = ps.tile(
                        [128, 512], F32, name=f"ctx{t}{n}", tag="ctx", bufs=1
                    )
                    ex_tiles = []

                    def ctx_mms(cp):
                        for j in range(NHQ):
                            mq = 17 * (NHQ * t + j)
                            nc.tensor.matmul(
                                ctx_t[32 * j : 32 * j + 17, :],
                                va[cp][:, mq : mq + 17],
                                ex_tiles[cp][:, 512 * j : 512 * (j + 1)],
                                start=(cp == 0),
                                stop=(cp == 7),
                                tile_position=(0, 32 * j),
                                skip_group_check=True,
                            )

                    for c in range(8):
                        sca = ps.tile(
                            [128, 1024], F32, name=f"sca{t}{n}{c}", tag="sc", bufs=3
                        )
                        scb = ps.tile(
                            [128, 1024], F32, name=f"scb{t}{n}{c}", tag="sc", bufs=3
                        )
                        for j in range(NHQ):
                            sch = sca if j < 2 else scb
                            nc.tensor.matmul(
                                sch[:, 512 * (j % 2) : 512 * (j % 2 + 1)],
                                kt[t][32 * j : 32 * (j + 1), 128 * c : 128 * (c + 1)],
                                qt[t][32 * j : 32 * (j + 1), 512 * n : 512 * (n + 1)],
                                start=True,
                                stop=True,
                                tile_position=(32 * j, 0),
                            )
                        ex = ap_.tile(
                            [128, 2048], BF16, name=f"ex{t}{n}{c}", tag="ex", bufs=4
                        )
                        nc.scalar.activation(ex[:, 0:1024], sca, AF.Exp, scale=SCALE)
                        nc.scalar.activation(
                            ex[:, 1024:2048], scb, AF.Exp, scale=SCALE
                        )
                        ex_tiles.append(ex)
                        # per-block PE fill work keeps the HAM warm:
                        if t == 0 and n == 0:
                            v_proj(c)
                        fill = PROJ_FILLS.get((t, n, c))
                        if fill is not None:
                            tau, h_, qk = fill
                            w_, x_, d_, b_ = (
                                (wq_all, xq_sb, qt, bq_sb)
                                if qk == "q"
                                else (wk_all, xk_sb, kt, bk_sb)
                            )
                            qk_proj_half(tau, h_, w_, x_, d_, b_, qk)
                        if t == NQUAD - 1 and n == 1 and c % 2 == 1:
                            outproj_group(
                                c // 2, 0, tag="aux" if c < 4 else "sc"
                            )
                        lag = 3 if (t == 0 and n == 0) else 1
                        if c >= lag:
                            ctx_mms(c - lag)
                    for cp in range(8 - lag, 8):
                        ctx_mms(cp)

                    # --- stage + denominators + normalize ---
                    st = st_bufs[stv % 2]
                    stv += 1
                    with tc.high_priority(offset=-160):
                        for j in range(NHQ):
                            nc.vector.tensor_copy(
                                st[32 * j : 32 * j + 17, :],
                                ctx_t[32 * j : 32 * j + 17, :],
                            )
                    rbw = ps.tile([128, 512], F32, name=f"rbw{t}{n}", tag="aux", bufs=1)
                    nc.tensor.matmul(rbw, sel_sb, st, start=True, stop=True)
                    rb = ap_.tile([128, 512], F32, name=f"rb{t}{n}", tag="rb", bufs=2)
                    rscr = ap_.tile(
                        [128, 512], F32, name=f"rs{t}{n}", tag="rscr", bufs=2
                    )
                    nc.vector.reciprocal_approx_accurate(rb, rbw, rscr)
                    for j in range(NHQ):
                        nc.vector.tensor_mul(
                            ctxp[t][32 * j : 32 * j + 16, 512 * n : 512 * (n + 1)],
                            st[32 * j : 32 * j + 16, :],
                            rb[32 * j : 32 * j + 16, :],
                        )

            # remaining output projection at the tail (exp stream done; sc
            # banks free, so po rotates across aux/ctx/sc)
            tags = ["aux", "ctx", "sc", "sc"]
            rest = [(m, 1) for m in range(4)]
            rest += [(m, dc) for m in range(4, 8) for dc in range(2)]
            for i, (m, dc) in enumerate(rest):
                outproj_group(
                    m, dc, tag=tags[i % 4],
                    evac="scalar" if i % 2 == 0 else "vector",
                )

    nc.finalize()
    return nc


# ---------------- host-side data prep ----------------

def _fp8(x):
    return np.ascontiguousarray(x, np.float32).astype(NP_FP8)


def _bf16(x):
    return np.ascontiguousarray(x, np.float32).astype(NP_BF16)


def prep_core_weights(g, Wq, bq, Wk, bk, Wv, bv, Wo):
    C0 = 256 * g

    def pack_w(W):
        # out[p, 4t+b, (pair, m=32j+e)] = W[C0+64t+16j+e, 256b+p+128*pair]
        Wg = W[C0 : C0 + 256, :]  # rows: 64t + 16j + e
        A = Wg.reshape(4, 4, 16, 4, 2, 128)  # [t, j, e, b, pair, p]
        out = np.zeros((128, 4, 4, 2, 4, 32), np.float32)  # [p, t, b, pair, j, m']
        out[..., :16] = A.transpose(5, 0, 3, 4, 1, 2)  # -> [p, t, b, pair, j, e]
        return _fp8(out.reshape(128, 16, 256))

    def pack_b(bvec):
        # [32j+e, t] = b[C0 + 64t + 16j + e], e < 16
        out = np.zeros((128, 4), np.float32)
        bg = bvec[C0 : C0 + 256].reshape(4, 4, 16)  # [t, j, e]
        for j in range(4):
            out[32 * j : 32 * j + 16, :] = bg[:, j].T
        return out

    wv = np.zeros((8, 128, VW), np.float32)  # [dblk, p, col] -> [p, dblk, col]
    bvrow = np.zeros((1, VW), np.float32)
    for m in range(16):
        src = C0 + 16 * m
        wvt = Wv[src : src + 16, :].T  # [1024, 16]
        wv[:, :, 17 * m : 17 * m + 16] = wvt.reshape(8, 128, 16)
        bvrow[0, 17 * m : 17 * m + 16] = bv[src : src + 16]
        bvrow[0, 17 * m + 16] = 1.0

    # wot[p, t, :]: p = 32j + e -> Wo[:, C0 + 64t + 16j + e], pad rows zero
    wot = np.zeros((128, 4, 1024), np.float32)
    for t in range(4):
        for j in range(4):
            src = C0 + 64 * t + 16 * j
            wot[32 * j : 32 * j + 16, t, :] = Wo[:, src : src + 16].T

    sel = np.zeros((128, 128), np.float32)
    for m in range(128):
        sel[32 * (m // 32) + 16, m] = 1.0

    return {
        "wq": pack_w(Wq),
        "wk": pack_w(Wk),
        "wv": _bf16(wv.transpose(1, 0, 2)),
        "wot": _bf16(wot),
        "bqp": pack_b(bq),
        "bkp": pack_b(bk),
        "bvrow": bvrow,
        "sel": round_fp32r(sel),
    }


def pack_x_fp8(Xb):
    # [128*sb + p, i, n] = Xb[n, 256*sb + 128*i + p]
    return _fp8(Xb.T.reshape(4, 2, 128, S).transpose(0, 2, 1, 3).reshape(512, 2, 1024))


def prep_in_maps(Q, K, V, Wq, bq, Wk, bk, Wv, bv, Wo):
    group_w = [prep_core_weights(g, Wq, bq, Wk, bk, Wv, bv, Wo) for g in range(4)]
    xt = []
    for b in range(B):
        xt.append(
            {
                "xq": pack_x_fp8(Q[b]),
                "xk": pack_x_fp8(K[b]),
                # xv[p, dblk, n] = V[b][n, 128*dblk + p]
                "xv": _bf16(V[b].T.reshape(8, 128, 1024).transpose(1, 0, 2)),
            }
        )
    in_maps = []
    for c in range(8):
        b, g = c // 4, c % 4
        m = dict(group_w[g])
        m.update(xt[b])
        in_maps.append(m)
    return in_maps


def assemble_output(results, bo):
    out = np.zeros((B, S, D), np.float32)
    for b in range(B):
        acc = np.zeros((S, D), np.float64)
        for g in range(4):
            acc += results[4 * b + g]["out_part"].astype(np.float64)
        out[b] = (acc + bo.astype(np.float64)).astype(np.float32)
    return out


_NC_CACHE = {}


def _get_nc():
    if "nc" not in _NC_CACHE:
        _NC_CACHE["nc"] = build_nc()
    return _NC_CACHE["nc"]


def kernel(Q, K, V, Wq, bq, Wk, bk, Wv, bv, Wo, bo):
    import time

    from concourse.bass_utils import run_bass_kernel_spmd

    nc = _get_nc()
    in_maps = prep_in_maps(
        np.asarray(Q, np.float32),
        np.asarray(K, np.float32),
        np.asarray(V, np.float32),
        np.asarray(Wq, np.float32),
        np.asarray(bq, np.float32),
        np.asarray(Wk, np.float32),
        np.asarray(bk, np.float32),
        np.asarray(Wv, np.float32),
        np.asarray(bv, np.float32),
        np.asarray(Wo, np.float32),
    )
    # Retries: a first execution after NEFF load occasionally hits a
    # transient NRT_EXEC_UNIT_UNRECOVERABLE; re-running recovers.
    last = None
    for attempt in range(3):
        try:
            res = run_bass_kernel_spmd(nc, in_maps, list(range(8)))
            return assemble_output(res.results, np.asarray(bo, np.float32))
        except Exception as e:
            last = e
            time.sleep(3)
    raise last


# revision 34
# speedup vs baseline: 1.0761x; 1.0761x over previous
"""Self-contained TRN2 Bass kernel for nn_MultiHeadAttention_77833397338481.

kernel(**inputs) takes the FULL unsharded inputs (Q, K, V [2,1024,1024],
Wq/Wk/Wv/Wo [1024,1024], biases [1024]) and returns the FULL output
[2, 1024, 1024]. 8 NeuronCores = batch(2) x head-group(4).

Per-core pipeline (balanced so the PE stays dense/warm while the scalar
engine's exp stream is the critical path):
  - q/k projections via fp8e4 DoubleRow matmuls (256-deep contraction per
    instruction), evacuated to packed f32r qt/kt (16 dims per 32-row band)
  - scores = k^T q per head as f32r matmuls with PE tile_position packing
  - exp on the scalar engine (PSUM f32 -> SBUF bf16), ones-column trick for
    softmax denominators, denominator broadcast via a select matmul
  - ctx and output projection in bf16; per-core partials summed on host
"""

import numpy as np
import ml_dtypes

import concourse.bacc as bacc
import concourse.mybir as mybir
import concourse.tile as tile

F32 = mybir.dt.float32
F32R = mybir.dt.float32r
BF16 = mybir.dt.bfloat16
FP8 = mybir.dt.float8e4
AF = mybir.ActivationFunctionType
DR = mybir.MatmulPerfMode.DoubleRow
ADD = mybir.AluOpType.add

NP_FP8 = ml_dtypes.float8_e4m3fn
NP_BF16 = ml_dtypes.bfloat16

D = 1024
S = 1024
B = 2
E = 16      # per-head dim
NHQ = 4     # heads per quad
NQUAD = 4   # quads per core
VW = 16 * 17  # 272: 16 heads x (16 v-dims + ones col)
SCALE = 1.0 / 32.0


def round_fp32r(x):
    u = np.ascontiguousarray(x, np.float32).view(np.uint32)
    r = ((u.astype(np.uint64) + 0x800) & 0xFFFFF000).astype(np.uint32)
    return r.view(np.float32)


def build_nc():
    nc = bacc.Bacc("TRN2", target_bir_lowering=False, debug=False, num_devices=8)

    xq_d = nc.dram_tensor("xq", [512, 2, 1024], FP8, kind="ExternalInput")
    xk_d = nc.dram_tensor("xk", [512, 2, 1024], FP8, kind="ExternalInput")
    xv_d = nc.dram_tensor("xv", [128, 8, 1024], BF16, kind="ExternalInput")
    wq_d = nc.dram_tensor("wq", [128, 16, 256], FP8, kind="ExternalInput")
    wk_d = nc.dram_tensor("wk", [128, 16, 256], FP8, kind="ExternalInput")
    wv_d = nc.dram_tensor("wv", [128, 8, VW], BF16, kind="ExternalInput")
    wot_d = nc.dram_tensor("wot", [128, 4, 1024], BF16, kind="ExternalInput")
    bq_d = nc.dram_tensor("bqp", [128, 4], F32, kind="ExternalInput")
    bk_d = nc.dram_tensor("bkp", [128, 4], F32, kind="ExternalInput")
    bvrow_d = nc.dram_tensor("bvrow", [1, VW], F32, kind="ExternalInput")
    sel_d = nc.dram_tensor("sel", [128, 128], F32R, kind="ExternalInput")
    out_d = nc.dram_tensor("out_part", [S, D], F32, kind="ExternalOutput")

    with tile.TileContext(nc) as tc:
        with (
            tc.tile_pool(name="persist", bufs=1) as pp,
            tc.tile_pool(name="attn", bufs=1) as ap_,
            tc.tile_pool(name="psum", space="PSUM", bufs=1) as ps,
        ):
            # --- warm the exp table ASAP ---
            dummy = pp.tile([1, 8], F32, name="dummy")
            nc.vector.memset(dummy, 0.0)
            dummy2 = pp.tile([1, 8], F32, name="dummy2")
            nc.scalar.activation(dummy2, dummy, AF.Exp)

            # --- constants ---
            sel_sb = pp.tile([128, 128], F32R, name="sel_sb")
            nc.sync.dma_start(out=sel_sb, in_=sel_d[:, :])
            bvrow_sb = pp.tile([1, VW], F32, name="bvrow_sb")
            nc.sync.dma_start(out=bvrow_sb, in_=bvrow_d[:, :])
            bq_sb = pp.tile([128, 4], F32, name="bq_sb")
            nc.sync.dma_start(out=bq_sb, in_=bq_d[:, :])
            bk_sb = pp.tile([128, 4], F32, name="bk_sb")
            nc.sync.dma_start(out=bk_sb, in_=bk_d[:, :])
            ones1 = pp.tile([1, 128], F32, name="ones1")
            nc.vector.memset(ones1, 1.0)

            # --- persistent tiles ---
            wq_all = pp.tile([128, 16, 256], FP8, name="wq_all")
            wk_all = pp.tile([128, 16, 256], FP8, name="wk_all")
            xq_sb = [pp.tile([128, 2, 1024], FP8, name=f"xq{b}") for b in range(4)]
            xk_sb = [pp.tile([128, 2, 1024], FP8, name=f"xk{b}") for b in range(4)]
            xv_all = pp.tile([128, 8, 1024], BF16, name="xv_all")
            wv_all = pp.tile([128, 8, VW], BF16, name="wv_all")
            wot_all = pp.tile([128, 4, 1024], BF16, name="wot_all")
            qt = [pp.tile([128, 1024], F32R, name=f"qt{t}") for t in range(NQUAD)]
            kt = [pp.tile([128, 1024], F32R, name=f"kt{t}") for t in range(NQUAD)]
            va = [pp.tile([128, VW], BF16, name=f"va{c}") for c in range(8)]
            # ctxp[t] rows 32j+e (e<16) hold head (t,j)'s normalized ctx; pad
            # rows must be zero (outproj contracts all 128 partitions)
            ctxp = [pp.tile([128, 1024], BF16, name=f"ctxp{t}") for t in range(NQUAD)]
            for t in range(NQUAD):
                nc.vector.memset(ctxp[t].bitcast(mybir.dt.uint16), 0)
            # persistent stage buffers, pre-zeroed (pad rows stay zero; the
            # select matmul contracts over them)
            st_bufs = [pp.tile([128, 512], F32R, name=f"stb{z}") for z in range(2)]
            for z in range(2):
                nc.vector.memset(st_bufs[z].bitcast(mybir.dt.uint32), 0)

            # --- input DMAs, in consumption order ---
            nc.sync.dma_start(out=wq_all, in_=wq_d[:, :, :])
            for b in range(4):
                nc.sync.dma_start(out=xq_sb[b], in_=xq_d[128 * b : 128 * (b + 1), :, :])
            nc.sync.dma_start(out=wk_all, in_=wk_d[:, :, :])
            for b in range(4):
                nc.sync.dma_start(out=xk_sb[b], in_=xk_d[128 * b : 128 * (b + 1), :, :])
            nc.sync.dma_start(out=wv_all, in_=wv_d[:, :, :])
            for c in range(8):
                nc.sync.dma_start(
                    out=xv_all[:, :, 128 * c : 128 * (c + 1)],
                    in_=xv_d[:, :, 128 * c : 128 * (c + 1)],
                )
            nc.sync.dma_start(out=wot_all, in_=wot_d[:, :, :])

            # --- q/k projection half: fp8 DoubleRow over 4 superblocks into
            # one aux bank, evacuated to packed f32r qt/kt ---
            def qk_proj_half(t, h, w_all, xs, dst, bias_sb, nm, tag="aux"):
                sc = ps.tile([128, 512], F32, name=f"p{nm}{t}{h}", tag=tag, bufs=1)
                for b in range(4):
                    wt = w_all[:, 4 * t + b : 4 * t + b + 1, :].rearrange(
                        "p o (pair m) -> p (o pair) m", pair=2
                    )
                    nc.tensor.matmul(
                        sc,
                        wt,
                        xs[b][:, :, 512 * h : 512 * (h + 1)],
                        start=(b == 0),
                        stop=(b == 3),
                        perf_mode=DR,
                    )
                nc.vector.tensor_scalar(
                    dst[t][:, 512 * h : 512 * (h + 1)],
                    sc,
                    bias_sb[:, t : t + 1],
                    None,
                    ADD,
                )

            # --- biasB = broadcast of bvrow over 128 partitions ---
            biasB_ps = ps.tile([128, VW], F32, name="biasB_ps", tag="sc", bufs=3)
            nc.tensor.matmul(biasB_ps, ones1, bvrow_sb, start=True, stop=True)
            biasB = pp.tile([128, VW], F32, name="biasB")
            nc.vector.tensor_copy(biasB, biasB_ps)

            # quad 0 h0 projections first so attention starts ASAP (h1
            # halves are spread into the first window's blocks)
            qk_proj_half(0, 0, wq_all, xq_sb, qt, bq_sb, "q")
            qk_proj_half(0, 0, wk_all, xk_sb, kt, bk_sb, "k")

            # --- v projection for one sk-block (spread into attention) ---
            def v_proj(c, tag="aux"):
                pv = ps.tile([128, VW], F32, name=f"pv{c}", tag=tag, bufs=1)
                for d in range(8):
                    nc.tensor.matmul(
                        pv,
                        xv_all[:, d : d + 1, 128 * c : 128 * (c + 1)],
                        wv_all[:, d : d + 1, :],
                        start=(d == 0),
                        stop=(d == 7),
                    )
                nc.vector.tensor_add(va[c], pv, biasB)


            # ================= attention =================
            def outproj_group(m, dc, tag="aux", evac="vector"):
                po = ps.tile(
                    [128, 512], F32, name=f"po{m}{dc}", tag=tag,
                    bufs=3 if tag == "sc" else 1,
                )
                for t_ in range(NQUAD):
                    nc.tensor.matmul(
                        po,
                        ctxp[t_][:, 128 * m : 128 * (m + 1)],
                        wot_all[:, t_ : t_ + 1, 512 * dc : 512 * (dc + 1)],
                        start=(t_ == 0),
                        stop=(t_ == NQUAD - 1),
                    )
                og = ap_.tile([128, 512], F32, name=f"og{m}{dc}", tag="og", bufs=4)
                if evac == "scalar":
                    nc.scalar.copy(og, po)
                else:
                    nc.vector.tensor_copy(og, po)
                nc.sync.dma_start(
                    out=out_d[128 * m : 128 * (m + 1), 512 * dc : 512 * (dc + 1)],
                    in_=og,
                )

            # projection-piece placement: (window t, n, block c) -> (quad, half, q/k)
            PROJ_FILLS = {
                (0, 0, 1): (0, 1, "k"),
                (0, 0, 2): (0, 1, "q"),
                (0, 1, 1): (1, 0, "q"),
                (0, 1, 3): (1, 0, "k"),
                (0, 1, 5): (1, 1, "k"),
                (0, 1, 7): (1, 1, "q"),
                (1, 0, 3): (2, 0, "q"),
                (1, 0, 5): (2, 0, "k"),
                (1, 1, 3): (2, 1, "k"),
                (1, 1, 5): (2, 1, "q"),
                (2, 0, 3): (3, 0, "q"),
                (2, 0, 5): (3, 0, "k"),
                (2, 1, 3): (3, 1, "k"),
                (2, 1, 5): (3, 1, "q"),
            }

            stv = 0
            for t in range(NQUAD):
                for n in range(2):
                    ctx_t = ps.tile(
                        [128, 512], F32, name=f"ctx{t}{n}", tag="ctx", bufs=1
                    )
                    ex_tiles = []

                    def ctx_mms(cp):
                        for j in range(NHQ):
                            mq = 17 * (NHQ * t + j)
                            nc.tensor.matmul(
                                ctx_t[32 * j : 32 * j + 17, :],
                                va[cp][:, mq : mq + 17],
                                ex_tiles[cp][:, 512 * j : 512 * (j + 1)],
                                start=(cp == 0),
                                stop=(cp == 7),
                                tile_position=(0, 32 * j),
                                skip_group_check=True,
                            )

                    for c in range(8):
                        sca = ps.tile(
                            [128, 1024], F32, name=f"sca{t}{n}{c}", tag="sc", bufs=3
                        )
                        scb = ps.tile(
                            [128, 1024], F32, name=f"scb{t}{n}{c}", tag="sc", bufs=3
                        )
                        for j in range(NHQ):
                            sch = sca if j < 2 else scb
                            nc.tensor.matmul(
                                sch[:, 512 * (j % 2) : 512 * (j % 2 + 1)],
                                kt[t][32 * j : 32 * (j + 1), 128 * c : 128 * (c + 1)],
                                qt[t][32 * j : 32 * (j + 1), 512 * n : 512 * (n + 1)],
                                start=True,
                                stop=True,
                                tile_position=(32 * j, 0),
                            )
                        ex = ap_.tile(
                            [128, 2048], BF16, name=f"ex{t}{n}{c}", tag="ex", bufs=4
                        )
                        nc.scalar.activation(ex[:, 0:1024], sca, AF.Exp, scale=SCALE)
                        nc.scalar.activation(
                            ex[:, 1024:2048], scb, AF.Exp, scale=SCALE
                        )
                        ex_tiles.append(ex)
                        # per-block PE fill work keeps the HAM warm:
                        if t == 0 and n == 0:
                            v_proj(c)
                        fill = PROJ_FILLS.get((t, n, c))
                        if fill is not None:
                            tau, h_, qk = fill
                            w_, x_, d_, b_ = (
                                (wq_all, xq_sb, qt, bq_sb)
                                if qk == "q"
                                else (wk_all, xk_sb, kt, bk_sb)
                            )
                            qk_proj_half(tau, h_, w_, x_, d_, b_, qk)
                        if t == NQUAD - 1 and n == 1 and c % 2 == 1:
                            outproj_group(
                                c // 2, 0, tag="aux" if c < 4 else "sc"
                            )
                        lag = 3 if (t == 0 and n == 0) else 1
                        if c >= lag:
                            ctx_mms(c - lag)
                    for cp in range(8 - lag, 8):
                        ctx_mms(cp)

                    # --- stage + denominators + normalize ---
                    st = st_bufs[stv % 2]
                    stv += 1
                    with tc.high_priority(offset=-160):
                        for j in range(NHQ):
                            nc.vector.tensor_copy(
                                st[32 * j : 32 * j + 17, :],
                                ctx_t[32 * j : 32 * j + 17, :],
                            )
                    rbw = ps.tile([128, 512], F32, name=f"rbw{t}{n}", tag="aux", bufs=1)
                    nc.tensor.matmul(rbw, sel_sb, st, start=True, stop=True)
                    rb = ap_.tile([128, 512], F32, name=f"rb{t}{n}", tag="rb", bufs=2)
                    rscr = ap_.tile(
                        [128, 512], F32, name=f"rs{t}{n}", tag="rscr", bufs=2
                    )
                    nc.vector.reciprocal_approx_accurate(rb, rbw, rscr)
                    for j in range(NHQ):
                        nc.vector.tensor_mul(
                            ctxp[t][32 * j : 32 * j + 16, 512 * n : 512 * (n + 1)],
                            st[32 * j : 32 * j + 16, :],
                            rb[32 * j : 32 * j + 16, :],
                        )

            # remaining output projection at the tail (exp stream done; sc
            # banks free, so po rotates across aux/ctx/sc)
            tags = ["aux", "ctx", "sc", "sc"]
            rest = [(m, 1) for m in range(4)]
            rest += [(m, dc) for m in range(4, 8) for dc in range(2)]
            for i, (m, dc) in enumerate(rest):
                outproj_group(
                    m, dc, tag=tags[i % 4],
                    evac="scalar" if i % 2 == 0 else "vector",
                )

    nc.finalize()
    return nc


# ---------------- host-side data prep ----------------

def _fp8(x):
    return np.ascontiguousarray(x, np.float32).astype(NP_FP8)


def _bf16(x):
    return np.ascontiguousarray(x, np.float32).astype(NP_BF16)


def prep_core_weights(g, Wq, bq, Wk, bk, Wv, bv, Wo):
    C0 = 256 * g

    def pack_w(W):
        # out[p, 4t+b, (pair, m=32j+e)] = W[C0+64t+16j+e, 256b+p+128*pair]
        Wg = W[C0 : C0 + 256, :]  # rows: 64t + 16j + e
        A = Wg.reshape(4, 4, 16, 4, 2, 128)  # [t, j, e, b, pair, p]
        out = np.zeros((128, 4, 4, 2, 4, 32), np.float32)  # [p, t, b, pair, j, m']
        out[..., :16] = A.transpose(5, 0, 3, 4, 1, 2)  # -> [p, t, b, pair, j, e]
        return _fp8(out.reshape(128, 16, 256))

    def pack_b(bvec):
        # [32j+e, t] = b[C0 + 64t + 16j + e], e < 16
        out = np.zeros((128, 4), np.float32)
        bg = bvec[C0 : C0 + 256].reshape(4, 4, 16)  # [t, j, e]
        for j in range(4):
            out[32 * j : 32 * j + 16, :] = bg[:, j].T
        return out

    wv = np.zeros((8, 128, VW), np.float32)  # [dblk, p, col] -> [p, dblk, col]
    bvrow = np.zeros((1, VW), np.float32)
    for m in range(16):
        src = C0 + 16 * m
        wvt = Wv[src : src + 16, :].T  # [1024, 16]
        wv[:, :, 17 * m : 17 * m + 16] = wvt.reshape(8, 128, 16)
        bvrow[0, 17 * m : 17 * m + 16] = bv[src : src + 16]
        bvrow[0, 17 * m + 16] = 1.0

    # wot[p, t, :]: p = 32j + e -> Wo[:, C0 + 64t + 16j + e], pad rows zero
    wot = np.zeros((128, 4, 1024), np.float32)
    for t in range(4):
        for j in range(4):
            src = C0 + 64 * t + 16 * j
            wot[32 * j : 32 * j + 16, t, :] = Wo[:, src : src + 16].T

    sel = np.zeros((128, 128), np.float32)
    for m in range(128):
        sel[32 * (m // 32) + 16, m] = 1.0

    return {
        "wq": pack_w(Wq),
        "wk": pack_w(Wk),
        "wv": _bf16(wv.transpose(1, 0, 2)),
        "wot": _bf16(wot),
        "bqp": pack_b(bq),
        "bkp": pack_b(bk),
        "bvrow": bvrow,
        "sel": round_fp32r(sel),
    }


def pack_x_fp8(Xb):
    # [128*sb + p, i, n] = Xb[n, 256*sb + 128*i + p]
    return _fp8(Xb.T.reshape(4, 2, 128, S).transpose(0, 2, 1, 3).reshape(512, 2, 1024))


def prep_in_maps(Q, K, V, Wq, bq, Wk, bk, Wv, bv, Wo):
    group_w = [prep_core_weights(g, Wq, bq, Wk, bk, Wv, bv, Wo) for g in range(4)]
    xt = []
    for b in range(B):
        xt.append(
            {
                "xq": pack_x_fp8(Q[b]),
                "xk": pack_x_fp8(K[b]),
                # xv[p, dblk, n] = V[b][n, 128*dblk + p]
                "xv": _bf16(V[b].T.reshape(8, 128, 1024).transpose(1, 0, 2)),
            }
        )
    in_maps = []
    for c in range(8):
        b, g = c // 4, c % 4
        m = dict(group_w[g])
        m.update(xt[b])
        in_maps.append(m)
    return in_maps


def assemble_output(results, bo):
    out = np.zeros((B, S, D), np.float32)
    for b in range(B):
        acc = np.zeros((S, D), np.float64)
        for g in range(4):
            acc += results[4 * b + g]["out_part"].astype(np.float64)
        out[b] = (acc + bo.astype(np.float64)).astype(np.float32)
    return out


_NC_CACHE = {}


def _get_nc():
    if "nc" not in _NC_CACHE:
        _NC_CACHE["nc"] = build_nc()
    return _NC_CACHE["nc"]


def kernel(Q, K, V, Wq, bq, Wk, bk, Wv, bv, Wo, bo):
    import time

    from concourse.bass_utils import run_bass_kernel_spmd

    nc = _get_nc()
    in_maps = prep_in_maps(
        np.asarray(Q, np.float32),
        np.asarray(K, np.float32),
        np.asarray(V, np.float32),
        np.asarray(Wq, np.float32),
        np.asarray(bq, np.float32),
        np.asarray(Wk, np.float32),
        np.asarray(bk, np.float32),
        np.asarray(Wv, np.float32),
        np.asarray(bv, np.float32),
        np.asarray(Wo, np.float32),
    )
    # Retries: a first execution after NEFF load occasionally hits a
    # transient NRT_EXEC_UNIT_UNRECOVERABLE; re-running recovers.
    last = None
    for attempt in range(3):
        try:
            res = run_bass_kernel_spmd(nc, in_maps, list(range(8)))
            return assemble_output(res.results, np.asarray(bo, np.float32))
        except Exception as e:
            last = e
            time.sleep(3)
    raise last


# revision 35
# speedup vs baseline: 1.0851x; 1.0084x over previous
"""Self-contained TRN2 Bass kernel for nn_MultiHeadAttention_77833397338481.

kernel(**inputs) takes the FULL unsharded inputs (Q, K, V [2,1024,1024],
Wq/Wk/Wv/Wo [1024,1024], biases [1024]) and returns the FULL output
[2, 1024, 1024]. 8 NeuronCores = batch(2) x head-group(4).

Per-core pipeline (balanced so the PE stays dense/warm while the scalar
engine's exp stream is the critical path):
  - q/k projections via fp8e4 DoubleRow matmuls (256-deep contraction per
    instruction), evacuated to packed f32r qt/kt (16 dims per 32-row band)
  - scores = k^T q per head as f32r matmuls with PE tile_position packing
  - exp on the scalar engine (PSUM f32 -> SBUF bf16), ones-column trick for
    softmax denominators, denominator broadcast via a select matmul
  - ctx and output projection in bf16; per-core partials summed on host
"""

import numpy as np
import ml_dtypes

import concourse.bacc as bacc
import concourse.mybir as mybir
import concourse.tile as tile

F32 = mybir.dt.float32
F32R = mybir.dt.float32r
BF16 = mybir.dt.bfloat16
FP8 = mybir.dt.float8e4
AF = mybir.ActivationFunctionType
DR = mybir.MatmulPerfMode.DoubleRow
ADD = mybir.AluOpType.add

NP_FP8 = ml_dtypes.float8_e4m3fn
NP_BF16 = ml_dtypes.bfloat16

D = 1024
S = 1024
B = 2
E = 16      # per-head dim
NHQ = 4     # heads per quad
NQUAD = 4   # quads per core
VW = 16 * 17  # 272: 16 heads x (16 v-dims + ones col)
SCALE = 1.0 / 32.0


def round_fp32r(x):
    u = np.ascontiguousarray(x, np.float32).view(np.uint32)
    r = ((u.astype(np.uint64) + 0x800) & 0xFFFFF000).astype(np.uint32)
    return r.view(np.float32)


def build_nc():
    nc = bacc.Bacc("TRN2", target_bir_lowering=False, debug=False, num_devices=8)

    xq_d = nc.dram_tensor("xq", [512, 2, 1024], FP8, kind="ExternalInput")
    xk_d = nc.dram_tensor("xk", [512, 2, 1024], FP8, kind="ExternalInput")
    xv_d = nc.dram_tensor("xv", [128, 8, 1024], BF16, kind="ExternalInput")
    wq_d = nc.dram_tensor("wq", [128, 16, 256], FP8, kind="ExternalInput")
    wk_d = nc.dram_tensor("wk", [128, 16, 256], FP8, kind="ExternalInput")
    wv_d = nc.dram_tensor("wv", [128, 8, VW], BF16, kind="ExternalInput")
    wot_d = nc.dram_tensor("wot", [128, 4, 1024], BF16, kind="ExternalInput")
    bq_d = nc.dram_tensor("bqp", [128, 4], F32, kind="ExternalInput")
    bk_d = nc.dram_tensor("bkp", [128, 4], F32, kind="ExternalInput")
    bvrow_d = nc.dram_tensor("bvrow", [1, VW], F32, kind="ExternalInput")
    sel_d = nc.dram_tensor("sel", [128, 128], F32R, kind="ExternalInput")
    out_d = nc.dram_tensor("out_part", [S, D], F32, kind="ExternalOutput")

    with tile.TileContext(nc) as tc:
        with (
            tc.tile_pool(name="persist", bufs=1) as pp,
            tc.tile_pool(name="attn", bufs=1) as ap_,
            tc.tile_pool(name="psum", space="PSUM", bufs=1) as ps,
        ):
            # --- warm the exp table ASAP ---
            dummy = pp.tile([1, 8], F32, name="dummy")
            nc.vector.memset(dummy, 0.0)
            dummy2 = pp.tile([1, 8], F32, name="dummy2")
            nc.scalar.activation(dummy2, dummy, AF.Exp)

            # --- constants ---
            sel_sb = pp.tile([128, 128], F32R, name="sel_sb")
            nc.sync.dma_start(out=sel_sb, in_=sel_d[:, :])
            bvrow_sb = pp.tile([1, VW], F32, name="bvrow_sb")
            nc.sync.dma_start(out=bvrow_sb, in_=bvrow_d[:, :])
            bq_sb = pp.tile([128, 4], F32, name="bq_sb")
            nc.sync.dma_start(out=bq_sb, in_=bq_d[:, :])
            bk_sb = pp.tile([128, 4], F32, name="bk_sb")
            nc.sync.dma_start(out=bk_sb, in_=bk_d[:, :])
            ones1 = pp.tile([1, 128], F32, name="ones1")
            nc.vector.memset(ones1, 1.0)

            # --- persistent tiles ---
            wq_all = pp.tile([128, 16, 256], FP8, name="wq_all")
            wk_all = pp.tile([128, 16, 256], FP8, name="wk_all")
            xq_sb = [pp.tile([128, 2, 1024], FP8, name=f"xq{b}") for b in range(4)]
            xk_sb = [pp.tile([128, 2, 1024], FP8, name=f"xk{b}") for b in range(4)]
            xv_all = pp.tile([128, 8, 1024], BF16, name="xv_all")
            wv_all = pp.tile([128, 8, VW], BF16, name="wv_all")
            wot_all = pp.tile([128, 4, 1024], BF16, name="wot_all")
            qt = [pp.tile([128, 1024], F32R, name=f"qt{t}") for t in range(NQUAD)]
            kt = [pp.tile([128, 1024], F32R, name=f"kt{t}") for t in range(NQUAD)]
            va = [pp.tile([128, VW], BF16, name=f"va{c}") for c in range(8)]
            # ctxp[t] rows 32j+e (e<16) hold head (t,j)'s normalized ctx; pad
            # rows must be zero (outproj contracts all 128 partitions)
            ctxp = [pp.tile([128, 1024], BF16, name=f"ctxp{t}") for t in range(NQUAD)]
            for t in range(NQUAD):
                nc.vector.memset(ctxp[t].bitcast(mybir.dt.uint16), 0)
            # persistent stage buffers, pre-zeroed (pad rows stay zero; the
            # select matmul contracts over them)
            st_bufs = [pp.tile([128, 512], F32R, name=f"stb{z}") for z in range(2)]
            for z in range(2):
                nc.vector.memset(st_bufs[z].bitcast(mybir.dt.uint32), 0)

            # --- input DMAs, in consumption order ---
            nc.sync.dma_start(out=wq_all, in_=wq_d[:, :, :])
            for b in range(4):
                nc.sync.dma_start(out=xq_sb[b], in_=xq_d[128 * b : 128 * (b + 1), :, :])
            nc.sync.dma_start(out=wk_all, in_=wk_d[:, :, :])
            for b in range(4):
                nc.sync.dma_start(out=xk_sb[b], in_=xk_d[128 * b : 128 * (b + 1), :, :])
            nc.sync.dma_start(out=wv_all, in_=wv_d[:, :, :])
            for c in range(8):
                nc.sync.dma_start(
                    out=xv_all[:, :, 128 * c : 128 * (c + 1)],
                    in_=xv_d[:, :, 128 * c : 128 * (c + 1)],
                )
            nc.sync.dma_start(out=wot_all, in_=wot_d[:, :, :])

            # --- q/k projection half: fp8 DoubleRow over 4 superblocks into
            # one aux bank, evacuated to packed f32r qt/kt ---
            def qk_proj_half(t, h, w_all, xs, dst, bias_sb, nm, tag="aux"):
                sc = ps.tile([128, 512], F32, name=f"p{nm}{t}{h}", tag=tag, bufs=1)
                for b in range(4):
                    wt = w_all[:, 4 * t + b : 4 * t + b + 1, :].rearrange(
                        "p o (pair m) -> p (o pair) m", pair=2
                    )
                    nc.tensor.matmul(
                        sc,
                        wt,
                        xs[b][:, :, 512 * h : 512 * (h + 1)],
                        start=(b == 0),
                        stop=(b == 3),
                        perf_mode=DR,
                    )
                nc.vector.tensor_scalar(
                    dst[t][:, 512 * h : 512 * (h + 1)],
                    sc,
                    bias_sb[:, t : t + 1],
                    None,
                    ADD,
                )

            # --- biasB = broadcast of bvrow over 128 partitions ---
            biasB_ps = ps.tile([128, VW], F32, name="biasB_ps", tag="sc", bufs=3)
            nc.tensor.matmul(biasB_ps, ones1, bvrow_sb, start=True, stop=True)
            biasB = pp.tile([128, VW], F32, name="biasB")
            nc.vector.tensor_copy(biasB, biasB_ps)

            # quad 0 h0 projections first so attention starts ASAP (h1
            # halves are spread into the first window's blocks)
            qk_proj_half(0, 0, wq_all, xq_sb, qt, bq_sb, "q")
            qk_proj_half(0, 0, wk_all, xk_sb, kt, bk_sb, "k")

            # --- v projection for one sk-block (spread into attention) ---
            def v_proj(c, tag="aux"):
                pv = ps.tile([128, VW], F32, name=f"pv{c}", tag=tag, bufs=1)
                for d in range(8):
                    nc.tensor.matmul(
                        pv,
                        xv_all[:, d : d + 1, 128 * c : 128 * (c + 1)],
                        wv_all[:, d : d + 1, :],
                        start=(d == 0),
                        stop=(d == 7),
                    )
                nc.vector.tensor_add(va[c], pv, biasB)


            # ================= attention =================
            def outproj_group(m, dc, tag="aux", evac="vector"):
                po = ps.tile(
                    [128, 512], F32, name=f"po{m}{dc}", tag=tag,
                    bufs=3 if tag == "sc" else 1,
                )
                for t_ in range(NQUAD):
                    nc.tensor.matmul(
                        po,
                        ctxp[t_][:, 128 * m : 128 * (m + 1)],
                        wot_all[:, t_ : t_ + 1, 512 * dc : 512 * (dc + 1)],
                        start=(t_ == 0),
                        stop=(t_ == NQUAD - 1),
                    )
                og = ap_.tile([128, 512], F32, name=f"og{m}{dc}", tag="og", bufs=4)
                if evac == "scalar":
                    nc.scalar.copy(og, po)
                else:
                    nc.vector.tensor_copy(og, po)
                nc.sync.dma_start(
                    out=out_d[128 * m : 128 * (m + 1), 512 * dc : 512 * (dc + 1)],
                    in_=og,
                )

            # projection-piece placement: (window t, n, block c) -> (quad, half, q/k)
            PROJ_FILLS = {
                (0, 0, 1): (0, 1, "k"),
                (0, 0, 2): (0, 1, "q"),
                (0, 1, 1): (1, 0, "q"),
                (0, 1, 3): (1, 0, "k"),
                (0, 1, 5): (1, 1, "k"),
                (0, 1, 7): (1, 1, "q"),
                (1, 0, 3): (2, 0, "q"),
                (1, 0, 5): (2, 0, "k"),
                (1, 1, 3): (2, 1, "k"),
                (1, 1, 5): (2, 1, "q"),
                (2, 0, 3): (3, 0, "q"),
                (2, 0, 5): (3, 0, "k"),
                (2, 1, 3): (3, 1, "k"),
                (2, 1, 5): (3, 1, "q"),
            }

            stv = 0
            for t in range(NQUAD):
                for n in range(2):
                    ctx_t = ps.tile(
                        [128, 512], F32, name=f"ctx{t}{n}", tag="ctx", bufs=1
                    )
                    ex_tiles = []

                    def ctx_mms(cp):
                        for j in range(NHQ):
                            mq = 17 * (NHQ * t + j)
                            nc.tensor.matmul(
                                ctx_t[32 * j : 32 * j + 17, :],
                                va[cp][:, mq : mq + 17],
                                ex_tiles[cp][:, 512 * j : 512 * (j + 1)],
                                start=(cp == 0),
                                stop=(cp == 7),
                                tile_position=(0, 32 * j),
                                skip_group_check=True,
                            )

                    for c in range(8):
                        sca = ps.tile(
                            [128, 1024], F32, name=f"sca{t}{n}{c}", tag="sc", bufs=3
                        )
                        scb = ps.tile(
                            [128, 1024], F32, name=f"scb{t}{n}{c}", tag="sc", bufs=3
                        )
                        for j in range(NHQ):
                            sch = sca if j < 2 else scb
                            nc.tensor.matmul(
                                sch[:, 512 * (j % 2) : 512 * (j % 2 + 1)],
                                kt[t][32 * j : 32 * (j + 1), 128 * c : 128 * (c + 1)],
                                qt[t][32 * j : 32 * (j + 1), 512 * n : 512 * (n + 1)],
                                start=True,
                                stop=True,
                                tile_position=(32 * j, 0),
                            )
                        ex = ap_.tile(
                            [128, 2048], BF16, name=f"ex{t}{n}{c}", tag="ex", bufs=6
                        )
                        nc.scalar.activation(ex[:, 0:1024], sca, AF.Exp, scale=SCALE)
                        nc.scalar.activation(
                            ex[:, 1024:2048], scb, AF.Exp, scale=SCALE
                        )
                        ex_tiles.append(ex)
                        # per-block PE fill work keeps the HAM warm:
                        if t == 0 and n == 0:
                            v_proj(c)
                        fill = PROJ_FILLS.get((t, n, c))
                        if fill is not None:
                            tau, h_, qk = fill
                            w_, x_, d_, b_ = (
                                (wq_all, xq_sb, qt, bq_sb)
                                if qk == "q"
                                else (wk_all, xk_sb, kt, bk_sb)
                            )
                            qk_proj_half(tau, h_, w_, x_, d_, b_, qk)
                        if t == NQUAD - 1 and n == 1 and c in (3, 5, 7):
                            outproj_group(
                                (c - 3) // 2, 0, tag="aux" if c > 3 else "sc"
                            )
                        lag = 3 if (t == 0 and n == 0) else 1
                        if c >= lag:
                            ctx_mms(c - lag)
                    for cp in range(8 - lag, 8):
                        ctx_mms(cp)

                    # --- stage + denominators + normalize ---
                    st = st_bufs[stv % 2]
                    stv += 1
                    with tc.high_priority(offset=-160):
                        for j in range(NHQ):
                            nc.vector.tensor_copy(
                                st[32 * j : 32 * j + 17, :],
                                ctx_t[32 * j : 32 * j + 17, :],
                            )
                    rbw = ps.tile([128, 512], F32, name=f"rbw{t}{n}", tag="aux", bufs=1)
                    nc.tensor.matmul(rbw, sel_sb, st, start=True, stop=True)
                    rb = ap_.tile([128, 512], F32, name=f"rb{t}{n}", tag="rb", bufs=2)
                    rscr = ap_.tile(
                        [128, 512], F32, name=f"rs{t}{n}", tag="rscr", bufs=2
                    )
                    nc.vector.reciprocal_approx_accurate(rb, rbw, rscr)
                    for j in range(NHQ):
                        nc.vector.tensor_mul(
                            ctxp[t][32 * j : 32 * j + 16, 512 * n : 512 * (n + 1)],
                            st[32 * j : 32 * j + 16, :],
                            rb[32 * j : 32 * j + 16, :],
                        )

            # remaining output projection at the tail (exp stream done; sc
            # banks free, so po rotates across aux/ctx/sc)
            tags = ["sc", "sc", "ctx", "aux"]
            rest = [(3, 0)] + [(m, 1) for m in range(4)]
            rest += [(m, dc) for m in range(4, 8) for dc in range(2)]
            for i, (m, dc) in enumerate(rest):
                outproj_group(
                    m, dc, tag=tags[i % 4],
                    evac="scalar" if i % 2 == 0 else "vector",
                )

    nc.finalize()
    return nc


# ---------------- host-side data prep ----------------

def _fp8(x):
    return np.ascontiguousarray(x, np.float32).astype(NP_FP8)


def _bf16(x):
    return np.ascontiguousarray(x, np.float32).astype(NP_BF16)


def prep_core_weights(g, Wq, bq, Wk, bk, Wv, bv, Wo):
    C0 = 256 * g

    def pack_w(W):
        # out[p, 4t+b, (pair, m=32j+e)] = W[C0+64t+16j+e, 256b+p+128*pair]
        Wg = W[C0 : C0 + 256, :]  # rows: 64t + 16j + e
        A = Wg.reshape(4, 4, 16, 4, 2, 128)  # [t, j, e, b, pair, p]
        out = np.zeros((128, 4, 4, 2, 4, 32), np.float32)  # [p, t, b, pair, j, m']
        out[..., :16] = A.transpose(5, 0, 3, 4, 1, 2)  # -> [p, t, b, pair, j, e]
        return _fp8(out.reshape(128, 16, 256))

    def pack_b(bvec):
        # [32j+e, t] = b[C0 + 64t + 16j + e], e < 16
        out = np.zeros((128, 4), np.float32)
        bg = bvec[C0 : C0 + 256].reshape(4, 4, 16)  # [t, j, e]
        for j in range(4):
            out[32 * j : 32 * j + 16, :] = bg[:, j].T
        return out

    wv = np.zeros((8, 128, VW), np.float32)  # [dblk, p, col] -> [p, dblk, col]
    bvrow = np.zeros((1, VW), np.float32)
    for m in range(16):
        src = C0 + 16 * m
        wvt = Wv[src : src + 16, :].T  # [1024, 16]
        wv[:, :, 17 * m : 17 * m + 16] = wvt.reshape(8, 128, 16)
        bvrow[0, 17 * m : 17 * m + 16] = bv[src : src + 16]
        bvrow[0, 17 * m + 16] = 1.0

    # wot[p, t, :]: p = 32j + e -> Wo[:, C0 + 64t + 16j + e], pad rows zero
    wot = np.zeros((128, 4, 1024), np.float32)
    for t in range(4):
        for j in range(4):
            src = C0 + 64 * t + 16 * j
            wot[32 * j : 32 * j + 16, t, :] = Wo[:, src : src + 16].T

    sel = np.zeros((128, 128), np.float32)
    for m in range(128):
        sel[32 * (m // 32) + 16, m] = 1.0

    return {
        "wq": pack_w(Wq),
        "wk": pack_w(Wk),
        "wv": _bf16(wv.transpose(1, 0, 2)),
        "wot": _bf16(wot),
        "bqp": pack_b(bq),
        "bkp": pack_b(bk),
        "bvrow": bvrow,
        "sel": round_fp32r(sel),
    }


def pack_x_fp8(Xb):
    # [128*sb + p, i, n] = Xb[n, 256*sb + 128*i + p]
    return _fp8(Xb.T.reshape(4, 2, 128, S).transpose(0, 2, 1, 3).reshape(512, 2, 1024))


def prep_in_maps(Q, K, V, Wq, bq, Wk, bk, Wv, bv, Wo):
    group_w = [prep_core_weights(g, Wq, bq, Wk, bk, Wv, bv, Wo) for g in range(4)]
    xt = []
    for b in range(B):
        xt.append(
            {
                "xq": pack_x_fp8(Q[b]),
                "xk": pack_x_fp8(K[b]),
                # xv[p, dblk, n] = V[b][n, 128*dblk + p]
                "xv": _bf16(V[b].T.reshape(8, 128, 1024).transpose(1, 0, 2)),
            }
        )
    in_maps = []
    for c in range(8):
        b, g = c // 4, c % 4
        m = dict(group_w[g])
        m.update(xt[b])
        in_maps.append(m)
    return in_maps


def assemble_output(results, bo):
    out = np.zeros((B, S, D), np.float32)
    for b in range(B):
        acc = np.zeros((S, D), np.float64)
        for g in range(4):
            acc += results[4 * b + g]["out_part"].astype(np.float64)
        out[b] = (acc + bo.astype(np.float64)).astype(np.float32)
    return out


_NC_CACHE = {}


def _get_nc():
    if "nc" not in _NC_CACHE:
        _NC_CACHE["nc"] = build_nc()
    return _NC_CACHE["nc"]


def kernel(Q, K, V, Wq, bq, Wk, bk, Wv, bv, Wo, bo):
    import time

    from concourse.bass_utils import run_bass_kernel_spmd

    nc = _get_nc()
    in_maps = prep_in_maps(
        np.asarray(Q, np.float32),
        np.asarray(K, np.float32),
        np.asarray(V, np.float32),
        np.asarray(Wq, np.float32),
        np.asarray(bq, np.float32),
        np.asarray(Wk, np.float32),
        np.asarray(bk, np.float32),
        np.asarray(Wv, np.float32),
        np.asarray(bv, np.float32),
        np.asarray(Wo, np.float32),
    )
    # Retries: a first execution after NEFF load occasionally hits a
    # transient NRT_EXEC_UNIT_UNRECOVERABLE; re-running recovers.
    last = None
    for attempt in range(3):
        try:
            res = run_bass_kernel_spmd(nc, in_maps, list(range(8)))
            return assemble_output(res.results, np.asarray(bo, np.float32))
        except Exception as e:
            last = e
            time.sleep(3)
    raise last
